# revision 46
# baseline (speedup 1.0000x reference)
"""Multi-head attention (SuperGlue-style, conv1x1 projections) on 8 Trainium2
NeuronCores.

Sharding: pure data-parallel over batch (B=8 -> 1 batch element per core),
zero collectives. Weights replicated.

Per-core structure (one batch element, x* = [D=256, N=2048], H=4, dh=64):
  q = 0.125 * (Wq x + bq), k = Wk x + bk          (PE convs, f32r)
  vT -> va[mc] [128, 512] bf16: per head [v_h(64) | ones(64)], so each
    head's num-matmul lhsT is one contiguous 128-col slice whose ones half
    makes PSUM rows 64-127 the softmax denominator REPLICATED 64x.
  attention, head-PAIR tiled: for tix in {0,1}, nt (4 x 512 n-window):
    per mc: S-pair (2 row-tiled K=64 matmuls, concurrent PE row groups),
      exp (one ACT instr [128,1024] -> bf16), num (2 K=128 matmuls into
      psN [128,1024], accumulated over all 16 m-chunks).
    m-chunks in OFF_MC run exp on DVE instead (bf16-domain Schraudolph:
      int16(A*s+B) bitcast as bf16, elem err ~3% diluted to ~1e-2 overall);
      their num matmuls are deferred to the window tail so the PE FIFO
      never stalls the ACT feed chain on the DVE round-trip.
    normalize: cross-base COPY den rows -> SBUF partitions 0-63 (the custom
      recip op corrupts at non-zero partition bases and is slow from PSUM,
      both HW-probed), reciprocal_approx_fast at base 0, then ONE same-base
      tensor_mul [64,1024] -> x_att (window-major layout).
  out = Wm' x_att + bm_eff    (bv folded into bm_eff; Wm bf16)

ACT does exp ONLY (112 x [128,1024] per rep, its HW roofline ~1us each);
every PSUM->SBUF move is on DVE, batched into [128,1024] ops; DMA goes
through sync (HWDGE, zero engine cost). va is double-buffered so the next
rep's V-proj copy doesn't wait on this rep's attention reads. PSUM:
psS 2 x 2 banks + psN 2 x 2 = 8 exactly (psN double-buffering decouples
the window-boundary normalize from the next window's accumulation).
"""

import numpy as np
from contextlib import ExitStack

import concourse.bass as bass
import concourse.tile as tile
from concourse import bacc, mybir
from concourse.bass_utils import run_bass_kernel_spmd

B, D, N, H = 8, 256, 2048, 4
DH = D // H            # 64 per-head channels
PC = 128               # partition chunk
KC = D // PC           # 2 contraction chunks for convs
NT = 512               # free-dim tile (fp32 matmul moving max)
NNT = N // NT          # 4 n-tiles
MC = N // PC           # 16 m-chunks (key/seq chunks on partitions)
VA_W = 512             # per-mc va stride: [v_h(64) | ones(64)] x 4 heads, so
                       # each head's num-matmul lhsT is ONE contiguous 128-col
                       # slice (matmul APs allow only one free dim).
F32 = mybir.dt.float32
F32R = mybir.dt.float32r
BF16 = mybir.dt.bfloat16
I16 = mybir.dt.int16

# Schraudolph exp in the bf16 domain: exp(x) ~= bitcast_bf16(int16(A*x + B)).
# Used on a few m-chunks per window to offload the ACT engine (elem err ~3%,
# diluted ~5x because only OFF_MC of 16 key-chunks per softmax are approx).
SCH_A = float(2 ** 7 / np.log(2))
SCH_B = float(127 * 2 ** 7) - 5.7
OFF_MC = ()            # ACT exp is ~160ns/op here; offload not worth it


def emit_consts(ctx: ExitStack, tc: tile.TileContext, io: dict):
    """Loop-invariant weight/bias/ones loads (outside the timing loop)."""
    nc = tc.nc
    consts = ctx.enter_context(tc.tile_pool(name="consts", bufs=1))
    c = {}
    c["w_q"] = [consts.tile([PC, D], F32R, tag=f"wq{kc}", name="wq") for kc in range(KC)]
    c["w_k"] = [consts.tile([PC, D], F32R, tag=f"wk{kc}", name="wk") for kc in range(KC)]
    c["w_v"] = [consts.tile([PC, D], F32R, tag=f"wv{kc}", name="wv") for kc in range(KC)]
    for kc in range(KC):
        nc.gpsimd.dma_start(c["w_q"][kc][:], io["wqT"][kc * PC:(kc + 1) * PC, :])
        nc.gpsimd.dma_start(c["w_k"][kc][:], io["wkT"][kc * PC:(kc + 1) * PC, :])
        nc.gpsimd.dma_start(c["w_v"][kc][:], io["wvT"][kc * PC:(kc + 1) * PC, :])
    c["w_m"] = [consts.tile([DH, D], BF16, tag=f"wm{h}", name="wm") for h in range(H)]
    for h in range(H):
        nc.gpsimd.dma_start(c["w_m"][h][:], io["wmT"][h * DH:(h + 1) * DH, :])
    ones_sb = consts.tile([PC, 1], F32R, tag="ones", name="ones")
    nc.gpsimd.dma_start(ones_sb[:], io["onec"].partition_broadcast(PC))
    # va: double-buffered [128, MC*VA_W] bf16; ones blocks written once here.
    va_pool = ctx.enter_context(tc.tile_pool(name="va", bufs=1))
    c["va"] = [va_pool.tile([PC, MC * VA_W], BF16, tag=f"va{i}", name="va")
               for i in range(2)]
    for i in range(2):
        for mc in range(MC):
            for h in range(H):
                dst = c["va"][i][:, mc * VA_W + h * 2 * DH + DH:
                                 mc * VA_W + (h + 1) * 2 * DH]
                nc.vector.tensor_copy(dst, ones_sb[:].broadcast_to([PC, DH]))
    c["b_q"] = [consts.tile([PC, 1], F32, tag=f"bq{oc}", name="bq") for oc in range(KC)]
    c["b_k"] = [consts.tile([PC, 1], F32, tag=f"bk{oc}", name="bk") for oc in range(KC)]
    c["b_m"] = [consts.tile([PC, 1], F32, tag=f"bm{oc}", name="bm") for oc in range(KC)]
    for oc in range(KC):
        nc.gpsimd.dma_start(c["b_q"][oc][:], io["bq"][oc * PC:(oc + 1) * PC, :])
        nc.gpsimd.dma_start(c["b_k"][oc][:], io["bk"][oc * PC:(oc + 1) * PC, :])
        nc.gpsimd.dma_start(c["b_m"][oc][:], io["bm"][oc * PC:(oc + 1) * PC, :])
    return c


def make_pools(ctx: ExitStack, tc: tile.TileContext):
    return {
        "in_pool": ctx.enter_context(tc.tile_pool(name="in_pool", bufs=6)),
        "qk_pool": ctx.enter_context(tc.tile_pool(name="qk_pool", bufs=4)),
        "e_pool": ctx.enter_context(tc.tile_pool(name="e_pool", bufs=4)),
        "x_pool": ctx.enter_context(tc.tile_pool(name="x_pool", bufs=2)),
        "sm_pool": ctx.enter_context(tc.tile_pool(name="sm_pool", bufs=2)),
        "out_pool": ctx.enter_context(tc.tile_pool(name="out_pool", bufs=2)),
        "psS": ctx.enter_context(tc.tile_pool(name="psS", bufs=2, space="PSUM")),
        "psN": ctx.enter_context(tc.tile_pool(name="psN", bufs=2, space="PSUM")),
    }


def emit_body(ctx: ExitStack, tc: tile.TileContext, io: dict, c: dict,
              pools: dict | None = None, body_idx: int = 0):
    nc = tc.nc
    xq, xk, xv = io["xq"], io["xk"], io["xv"]
    out = io["out"]
    w_q, w_k, w_v, w_m = c["w_q"], c["w_k"], c["w_v"], c["w_m"]
    b_q, b_k, b_m = c["b_q"], c["b_k"], c["b_m"]
    va = c["va"][body_idx % 2]

    if pools is None:
        pools = make_pools(ctx, tc)
    in_pool = pools["in_pool"]; qk_pool = pools["qk_pool"]
    e_pool = pools["e_pool"]
    x_pool = pools["x_pool"]; sm_pool = pools["sm_pool"]
    out_pool = pools["out_pool"]; psS = pools["psS"]; psN = pools["psN"]

    # --- load activations (sync engine -> HWDGE, no engine cost) ---
    x_in = {}
    for name, dram in (("xq", xq), ("xk", xk), ("xv", xv)):
        x_in[name] = [in_pool.tile([PC, N], F32R, tag="xin", name="xin") for _ in range(KC)]
        for kc in range(KC):
            nc.sync.dma_start(x_in[name][kc][:], dram[kc * PC:(kc + 1) * PC, :])

    # --- Q / K projections (bf16 out; PSUM->SBUF moves batched 1024-wide) ---
    q_sb = [qk_pool.tile([PC, N], BF16, tag="qsb", name="qsb") for _ in range(KC)]
    k_sb = [qk_pool.tile([PC, N], BF16, tag="ksb", name="ksb") for _ in range(KC)]
    for w_sb, b_sb, x_sb_in, dst in (
        (w_q, b_q, x_in["xq"], q_sb),
        (w_k, b_k, x_in["xk"], k_sb),
    ):
        for oc in range(KC):
            for nt2 in range(NNT // 2):
                ps = psS.tile([PC, 2 * NT], F32, tag="sps", name="cps")
                for half in range(2):
                    nt = nt2 * 2 + half
                    for kc in range(KC):
                        nc.tensor.matmul(
                            ps[:, half * NT:(half + 1) * NT],
                            lhsT=w_sb[kc][:, oc * PC:(oc + 1) * PC],
                            rhs=x_sb_in[kc][:, nt * NT:(nt + 1) * NT],
                            start=(kc == 0),
                            stop=(kc == KC - 1),
                        )
                nc.vector.tensor_scalar_add(
                    dst[oc][:, nt2 * 2 * NT:(nt2 + 1) * 2 * NT], ps[:], b_sb[oc][:]
                )

    # --- V^T projection: 4 m-chunks per PSUM tile, one strided DVE copy ---
    for g in range(MC // 4):
        ps = psS.tile([PC, 2 * NT], F32, tag="sps", name="cps")
        for sub in range(4):
            mc = g * 4 + sub
            for kc in range(KC):
                nc.tensor.matmul(
                    ps[:, sub * D:(sub + 1) * D],
                    lhsT=x_in["xv"][kc][:, mc * PC:(mc + 1) * PC],
                    rhs=w_v[kc][:],
                    start=(kc == 0),
                    stop=(kc == KC - 1),
                )
        dst = (va[:, g * 4 * VA_W:(g + 1) * 4 * VA_W]
               .rearrange("p (k c) -> p k c", k=16)[:, :, 0:DH])
        src = ps[:].rearrange("p (k c) -> p k c", k=16)
        nc.scalar.activation(dst, src, mybir.ActivationFunctionType.Copy)

    # --- attention, head-pair tiled; x_att window-major [64, 8*1024] ---
    x_att = x_pool.tile([DH, 2 * NNT * 2 * NT], BF16, tag="xatt", name="xatt")
    for tix in range(H // 2):          # head pair (2*tix, 2*tix+1)
        for nt in range(NNT):
            nps = psN.tile([PC, 2 * NT], F32, tag="nps", name="nps")

            def emit_num(mc, e_t, start, stop):
                for j in range(2):
                    h = tix * 2 + j
                    a = mc * VA_W + h * 2 * DH  # [v_h(64) | ones(64)] slice
                    nc.tensor.matmul(
                        nps[:, j * NT:(j + 1) * NT],
                        lhsT=va[:, a:a + 2 * DH],
                        rhs=e_t[:, j * NT:(j + 1) * NT],
                        start=start,
                        stop=stop,
                    )

            deferred = []
            for mc in range(MC):
                sps = psS.tile([PC, 2 * NT], F32, tag="sps", name="sps")
                for j in range(2):     # local head index within the pair
                    hb = j * DH
                    nc.tensor.matmul(
                        sps[:, j * NT:(j + 1) * NT],
                        lhsT=k_sb[tix][hb:hb + DH, mc * PC:(mc + 1) * PC],
                        rhs=q_sb[tix][hb:hb + DH, nt * NT:(nt + 1) * NT],
                        start=True,
                        stop=True,
                    )
                if mc in OFF_MC:
                    # DVE Schraudolph exp; its num matmuls are DEFERRED to
                    # the window tail so the PE FIFO never stalls the ACT
                    # feed chain on the DVE round-trip.
                    e_i = e_pool.tile([PC, 2 * NT], I16, tag="eti", name="eti")
                    nc.vector.tensor_scalar(
                        e_i[:], sps[:], SCH_A, SCH_B,
                        mybir.AluOpType.mult, mybir.AluOpType.add,
                    )
                    deferred.append((mc, e_i[:].bitcast(BF16)))
                    continue
                e_b = e_pool.tile([PC, 2 * NT], BF16, tag="et", name="et")
                nc.scalar.activation(e_b[:], sps[:],
                                     mybir.ActivationFunctionType.Exp)
                last_plain = not OFF_MC and mc == MC - 1
                emit_num(mc, e_b[:], start=(mc == 0), stop=last_plain)
            for i, (mc, e_t) in enumerate(deferred):
                emit_num(mc, e_t, start=False, stop=(i == len(deferred) - 1))
            # normalize both heads of this n-window: rows 64-127 of nps
            # hold the denominator replicated 64x. reciprocal_approx_fast
            # corrupts at non-zero partition bases and is slow reading PSUM
            # (both HW-probed), so: cross-base COPY the denominator down to
            # SBUF partitions 0-63, recip at base 0 from SBUF, then a
            # same-base mul into the window-major x_att slot.
            den = sm_pool.tile([DH, 2 * NT], F32, tag="den", name="den")
            nc.vector.tensor_copy(den[:], nps[DH:PC, :])
            rb = sm_pool.tile([DH, 2 * NT], F32, tag="recip", name="recip")
            with nc.allow_low_precision(reason="recip of softmax denom"):
                nc.vector.reciprocal_approx_fast(rb[:], den[:])
                w = tix * NNT + nt
                nc.vector.tensor_mul(
                    x_att[:, w * 2 * NT:(w + 1) * 2 * NT],
                    nps[0:DH, :],
                    rb[:],
                )

    # --- merge projection (rhs via window-major x_att slices) ---
    for oc in range(KC):
        o_t = out_pool.tile([PC, N], BF16, tag="ot", name="ot")
        for nt2 in range(NNT // 2):
            ps = psS.tile([PC, 2 * NT], F32, tag="sps", name="cps")
            for half in range(2):
                nt = nt2 * 2 + half
                for h in range(H):
                    tix, j = h // 2, h % 2
                    w = tix * NNT + nt
                    nc.tensor.matmul(
                        ps[:, half * NT:(half + 1) * NT],
                        lhsT=w_m[h][:, oc * PC:(oc + 1) * PC],
                        rhs=x_att[:, w * 2 * NT + j * NT:w * 2 * NT + (j + 1) * NT],
                        start=(h == 0),
                        stop=(h == H - 1),
                    )
            nc.vector.tensor_scalar_add(
                o_t[:, nt2 * 2 * NT:(nt2 + 1) * 2 * NT], ps[:], b_m[oc][:]
            )
        nc.sync.dma_start(out[oc * PC:(oc + 1) * PC, :], o_t[:])


def build_nc(reps=1):
    nc = bacc.Bacc("TRN2", target_bir_lowering=False, debug=False, num_devices=B)
    io = {
        "xq": nc.dram_tensor("xq", [D, N], F32R, kind="ExternalInput").ap(),
        "xk": nc.dram_tensor("xk", [D, N], F32R, kind="ExternalInput").ap(),
        "xv": nc.dram_tensor("xv", [D, N], F32R, kind="ExternalInput").ap(),
        "wqT": nc.dram_tensor("wqT", [D, D], F32R, kind="ExternalInput").ap(),
        "wkT": nc.dram_tensor("wkT", [D, D], F32R, kind="ExternalInput").ap(),
        "wvT": nc.dram_tensor("wvT", [D, D], F32R, kind="ExternalInput").ap(),
        "wmT": nc.dram_tensor("wmT", [D, D], BF16, kind="ExternalInput").ap(),
        "bq": nc.dram_tensor("bq", [D, 1], F32, kind="ExternalInput").ap(),
        "bk": nc.dram_tensor("bk", [D, 1], F32, kind="ExternalInput").ap(),
        "bm": nc.dram_tensor("bm", [D, 1], F32, kind="ExternalInput").ap(),
        "onec": nc.dram_tensor("onec", [1, 1], F32R, kind="ExternalInput").ap(),
        "out": nc.dram_tensor("out", [D, N], BF16, kind="ExternalOutput").ap(),
    }
    with tile.TileContext(nc) as tc:
        with ExitStack() as ctx:
            c = emit_consts(ctx, tc, io)
            if reps == 1:
                with ExitStack() as ctx2:
                    emit_body(ctx2, tc, io, c)
            elif reps % 4 == 0:
                with tc.For_i(0, reps // 4, 1):
                    with ExitStack() as ctx2:
                        pools = make_pools(ctx2, tc)
                        for i in range(4):
                            emit_body(ctx2, tc, io, c, pools, body_idx=i)
            elif reps % 2 == 0:
                with tc.For_i(0, reps // 2, 1):
                    with ExitStack() as ctx2:
                        pools = make_pools(ctx2, tc)
                        for i in range(2):
                            emit_body(ctx2, tc, io, c, pools, body_idx=i)
            else:
                with tc.For_i(0, reps, 1):
                    with ExitStack() as ctx2:
                        emit_body(ctx2, tc, io, c)
    nc.compile()
    return nc


def host_inputs(query, key, value, Wq, bq, Wk, bk, Wv, bv, Wm, bm):
    """Host-side prep: head-deinterleaving permutation + scale/bias folding.

    Returns (shared weight map, list of per-core input maps)."""
    from ml_dtypes import bfloat16

    f = np.float32
    t = np.arange(D)
    perm = (t % DH) * H + t // DH  # row t = head-major channel -> original dm

    Wq = np.asarray(Wq, f); Wk = np.asarray(Wk, f); Wv = np.asarray(Wv, f)
    Wm = np.asarray(Wm, f)
    bq = np.asarray(bq, f); bk = np.asarray(bk, f); bv = np.asarray(bv, f)
    bm = np.asarray(bm, f)

    scale = f(1.0 / np.sqrt(DH))
    shared = {
        "onec": np.ones((1, 1), f),
        "wqT": np.ascontiguousarray(Wq.T[:, perm] * scale),
        "wkT": np.ascontiguousarray(Wk.T[:, perm]),
        "wvT": np.ascontiguousarray(Wv.T[:, perm]),
        "wmT": np.ascontiguousarray(Wm.T[perm, :]).astype(bfloat16),
        "bq": np.ascontiguousarray((bq[perm] * scale).reshape(D, 1)),
        "bk": np.ascontiguousarray(bk[perm].reshape(D, 1)),
        "bm": np.ascontiguousarray((bm + Wm @ bv).reshape(D, 1)),
    }
    query = np.asarray(query, f); key = np.asarray(key, f)
    value = np.asarray(value, f)
    in_maps = []
    for b in range(B):
        m = dict(shared)
        m["xq"] = np.ascontiguousarray(query[b])
        m["xk"] = np.ascontiguousarray(key[b])
        m["xv"] = np.ascontiguousarray(value[b])
        in_maps.append(m)
    return shared, in_maps


_NC = None


def get_nc():
    global _NC
    if _NC is None:
        _NC = build_nc()
    return _NC


def kernel(query, key, value, Wq, bq, Wk, bk, Wv, bv, Wm, bm):
    nc = get_nc()
    _, in_maps = host_inputs(query, key, value, Wq, bq, Wk, bk, Wv, bv, Wm, bm)
    res = run_bass_kernel_spmd(nc, in_maps, core_ids=list(range(B)))
    return np.stack([np.asarray(res.results[b]["out"], np.float32)
                     for b in range(B)], axis=0)
